# revision 5
# baseline (speedup 1.0000x reference)
"""Trainium2 Bass kernel for DBFLinear:
    y = ((x * s0) @ unpack(bp1).T * s2) @ unpack(bp3).T * s4 + bias

Strategy: data-parallel over batch across 8 cores (weights replicated, no
collectives). Per core: unpack the bit-packed +/-1 weights on device
(DVE bitwise_and + ACT Sign), transpose weight blocks with the DMA xbar,
run both GEMMs weight-stationary (fp16, fp32 PSUM accumulation), fold all
scalings/bias into per-partition ACT ops. The device emits y.T per batch
shard; the host transposes while unsharding.
"""

import sys

import numpy as np

sys.path.insert(0, "/opt/trn_rl_repo")

import concourse.bass as bass
import concourse.mybir as mybir
import concourse.tile as tile
from concourse import bacc
from concourse.bass_utils import run_bass_kernel_spmd

N_CORES = 8
B_FULL, IN, MID, OUT = 8192, 4096, 4096, 4096
P = 128
FD = 512  # matmul moving-operand free dim (1 PSUM bank of fp32)
UCH = 2048  # unpack chunk width (weight elements per DVE/ACT op)


def build_program(b=B_FULL // N_CORES, in_=IN, mid=MID, out=OUT):
    """Build the per-core Bass program. Returns the Bass object."""
    in_k, mid_k, out_k = in_ // P, mid // P, out // P
    nbc = max(1, b // FD)
    fd = min(FD, b)
    uch = min(UCH, in_, mid)

    nc = bacc.Bacc(num_devices=N_CORES)
    x_d = nc.dram_tensor("x", [b, in_], mybir.dt.float16, kind="ExternalInput")
    bp1_d = nc.dram_tensor("bp1", [mid, in_ // 8], mybir.dt.int32, kind="ExternalInput")
    bp3_d = nc.dram_tensor("bp3", [out, mid // 8], mybir.dt.int32, kind="ExternalInput")
    mask_d = nc.dram_tensor("mask", [P, 8], mybir.dt.int32, kind="ExternalInput")
    s0_d = nc.dram_tensor("s0", [P, in_k], mybir.dt.float32, kind="ExternalInput")
    s2_d = nc.dram_tensor("s2", [P, mid_k], mybir.dt.float32, kind="ExternalInput")
    s4_d = nc.dram_tensor("s4", [P, out_k], mybir.dt.float32, kind="ExternalInput")
    bias_d = nc.dram_tensor("bias", [P, out_k], mybir.dt.float32, kind="ExternalInput")
    yT_d = nc.dram_tensor("yT", [out, b], mybir.dt.float16, kind="ExternalOutput")

    Act = mybir.ActivationFunctionType

    with tile.TileContext(nc) as tc:
        with (
            tc.tile_pool(name="big", bufs=1) as big,
            tc.tile_pool(name="consts", bufs=1) as consts,
            tc.tile_pool(name="wpipe", bufs=2) as wpipe,
            tc.tile_pool(name="psum", bufs=4, space="PSUM") as psum,
        ):
            mask_t = consts.tile([P, 8], mybir.dt.int32)
            s0_t = consts.tile([P, in_k], mybir.dt.float32)
            s2_t = consts.tile([P, mid_k], mybir.dt.float32)
            s4_t = consts.tile([P, out_k], mybir.dt.float32)
            bias_t = consts.tile([P, out_k], mybir.dt.float32)
            neg_half = consts.tile([P, 1], mybir.dt.float32)
            for t, d in (
                (mask_t, mask_d),
                (s0_t, s0_d),
                (s2_t, s2_d),
                (s4_t, s4_d),
                (bias_t, bias_d),
            ):
                nc.sync.dma_start(t[:], d[:])
            nc.vector.memset(neg_half[:], -0.5)

            # x.T: xT[p, k, r] = x[r, 128k + p], then scale by s0 per partition.
            # One transpose per k-block keeps the per-instruction xbar tile
            # count (and thus DMA semaphore thresholds) small.
            xT = big.tile([P, in_k, b], mybir.dt.float16)
            for k in range(in_k):
                nc.sync.dma_start_transpose(
                    xT[:, k, :], x_d[:, k * P : (k + 1) * P]
                )
                nc.scalar.activation(
                    xT[:, k, :], xT[:, k, :], Act.Copy, scale=s0_t[:, k : k + 1]
                )

            hT = big.tile([P, mid_k, b], mybir.dt.float16)

            def unpack_wT(bp_d, m, k_blocks):
                """Unpack 128 rows (block m) of a packed sign matrix and
                return its transposed [P, k_blocks, P] weight tile."""
                kb = k_blocks * P // 8  # bytes per row
                byt = wpipe.tile([P, kb], mybir.dt.int32, tag="bytes")
                nc.sync.dma_start(byt[:], bp_d[m * P : (m + 1) * P, :])
                w_nat = wpipe.tile([P, k_blocks * P], mybir.dt.float16, tag="wnat")
                for c0 in range(0, k_blocks * P, uch):
                    nb = uch // 8
                    b0 = c0 // 8
                    masked = wpipe.tile([P, uch], mybir.dt.int32, tag="masked")
                    in0 = byt[:, b0 : b0 + nb][:, :, None].broadcast_to([P, nb, 8])
                    in1 = mask_t[:][:, None, :].broadcast_to([P, nb, 8])
                    nc.vector.tensor_tensor(
                        masked[:].rearrange("p (b j) -> p b j", j=8),
                        in0,
                        in1,
                        mybir.AluOpType.bitwise_and,
                    )
                    nc.scalar.activation(
                        w_nat[:, c0 : c0 + uch],
                        masked[:],
                        Act.Sign,
                        bias=neg_half[:, 0:1],
                    )
                wT = wpipe.tile([P, k_blocks, P], mybir.dt.float16, tag="wT")
                nc.sync.dma_start_transpose(wT[:], w_nat[:])
                return wT

            # GEMM1: hT[mid, b] = W1 @ xT, scaled by s2
            for m in range(mid_k):
                wT = unpack_wT(bp1_d, m, in_k)
                for c in range(nbc):
                    ps = psum.tile([P, fd], mybir.dt.float32, tag="ps")
                    for k in range(in_k):
                        nc.tensor.matmul(
                            ps[:],
                            wT[:, k, :],
                            xT[:, k, c * fd : (c + 1) * fd],
                            start=(k == 0),
                            stop=(k == in_k - 1),
                        )
                    nc.scalar.activation(
                        hT[:, m, c * fd : (c + 1) * fd],
                        ps[:],
                        Act.Copy,
                        scale=s2_t[:, m : m + 1],
                    )

            # GEMM2: yT[out, b] = W3 @ hT, scaled by s4 plus bias
            for o in range(out_k):
                wT = unpack_wT(bp3_d, o, mid_k)
                yt = wpipe.tile([P, b], mybir.dt.float16, tag="yt")
                for c in range(nbc):
                    ps = psum.tile([P, fd], mybir.dt.float32, tag="ps")
                    for k in range(mid_k):
                        nc.tensor.matmul(
                            ps[:],
                            wT[:, k, :],
                            hT[:, k, c * fd : (c + 1) * fd],
                            start=(k == 0),
                            stop=(k == mid_k - 1),
                        )
                    nc.scalar.activation(
                        yt[:, c * fd : (c + 1) * fd],
                        ps[:],
                        Act.Identity,
                        bias=bias_t[:, o : o + 1],
                        scale=s4_t[:, o : o + 1],
                    )
                nc.sync.dma_start(yT_d[o * P : (o + 1) * P, :], yt[:])

    nc.compile()
    return nc


def make_in_maps(x, scaling0, bp1, scaling2, bp3, scaling4, bias, n_cores=N_CORES):
    b_full, in_ = x.shape
    mid = scaling2.shape[0]
    out = scaling4.shape[0]
    b = b_full // n_cores

    mask = (1 << (7 - np.arange(8, dtype=np.int32)))[None, :].repeat(P, 0)
    mask = np.ascontiguousarray(mask.astype(np.int32))

    def pcol(v):
        return np.ascontiguousarray(v.astype(np.float32).reshape(-1, P).T)

    shared = {
        "bp1": np.ascontiguousarray(bp1.reshape(mid, in_ // 8)),
        "bp3": np.ascontiguousarray(bp3.reshape(out, mid // 8)),
        "mask": mask,
        "s0": pcol(scaling0),
        "s2": pcol(scaling2),
        "s4": pcol(scaling4),
        "bias": pcol(bias),
    }
    return [
        {"x": np.ascontiguousarray(x[c * b : (c + 1) * b]), **shared}
        for c in range(n_cores)
    ]


_PROGRAM_CACHE = {}


def run(x, scaling0, bp1, scaling2, bp3, scaling4, bias, **spmd_kwargs):
    """Compile (cached) + run on 8 cores; returns (y, BassKernelResults)."""
    if "nc" not in _PROGRAM_CACHE:
        _PROGRAM_CACHE["nc"] = build_program()
    nc = _PROGRAM_CACHE["nc"]
    in_maps = make_in_maps(x, scaling0, bp1, scaling2, bp3, scaling4, bias)
    res = run_bass_kernel_spmd(nc, in_maps, core_ids=list(range(N_CORES)), **spmd_kwargs)
    b = x.shape[0] // N_CORES
    y = np.empty((x.shape[0], scaling4.shape[0]), dtype=np.float16)
    for c in range(N_CORES):
        y[c * b : (c + 1) * b] = res.results[c]["yT"].T
    return y, res


def kernel(x, scaling0, bp1, scaling2, bp3, scaling4, bias):
    y, _ = run(x, scaling0, bp1, scaling2, bp3, scaling4, bias)
    return y


# revision 7
# speedup vs baseline: 1.0157x; 1.0157x over previous
"""Trainium2 Bass kernel for DBFLinear:
    y = ((x * s0) @ unpack(bp1).T * s2) @ unpack(bp3).T * s4 + bias

Strategy: data-parallel over batch across 8 cores (weights replicated, no
collectives). Per core: unpack the bit-packed +/-1 weights on device
(DVE bitwise_and + ACT Sign), transpose weight blocks with the DMA xbar,
run both GEMMs weight-stationary (fp16, fp32 PSUM accumulation), fold all
scalings/bias into per-partition ACT ops. The device emits y.T per batch
shard; the host transposes while unsharding.
"""

import sys

import numpy as np

sys.path.insert(0, "/opt/trn_rl_repo")

import concourse.bass as bass
import concourse.mybir as mybir
import concourse.tile as tile
from concourse import bacc
from concourse.bass_utils import run_bass_kernel_spmd

N_CORES = 8
B_FULL, IN, MID, OUT = 8192, 4096, 4096, 4096
P = 128
FD = 512  # matmul moving-operand free dim (1 PSUM bank of fp32)
UCH = 2048  # unpack chunk width (weight elements per DVE/ACT op)


def build_program(b=B_FULL // N_CORES, in_=IN, mid=MID, out=OUT):
    """Build the per-core Bass program. Returns the Bass object."""
    in_k, mid_k, out_k = in_ // P, mid // P, out // P
    nbc = 2  # batch processed as two halves
    fd = b // nbc
    assert fd <= FD, (b, fd)
    uch = min(UCH, in_, mid)

    nc = bacc.Bacc(num_devices=N_CORES)
    x_d = nc.dram_tensor("x", [b, in_], mybir.dt.float16, kind="ExternalInput")
    bp1_d = nc.dram_tensor("bp1", [mid, in_ // 8], mybir.dt.int32, kind="ExternalInput")
    bp3_d = nc.dram_tensor("bp3", [out, mid // 8], mybir.dt.int32, kind="ExternalInput")
    mask_d = nc.dram_tensor("mask", [P, 8], mybir.dt.int32, kind="ExternalInput")
    s0_d = nc.dram_tensor("s0", [P, in_k], mybir.dt.float32, kind="ExternalInput")
    s2_d = nc.dram_tensor("s2", [P, mid_k], mybir.dt.float32, kind="ExternalInput")
    s4_d = nc.dram_tensor("s4", [P, out_k], mybir.dt.float32, kind="ExternalInput")
    bias_d = nc.dram_tensor("bias", [P, out_k], mybir.dt.float32, kind="ExternalInput")
    yT_d = nc.dram_tensor("yT", [out, b], mybir.dt.float16, kind="ExternalOutput")

    Act = mybir.ActivationFunctionType

    with tile.TileContext(nc) as tc:
        with (
            tc.tile_pool(name="big", bufs=1) as big,
            tc.tile_pool(name="consts", bufs=1) as consts,
            tc.tile_pool(name="wpipe", bufs=2) as wpipe,
            tc.tile_pool(name="psum", bufs=4, space="PSUM") as psum,
        ):
            mask_t = consts.tile([P, 8], mybir.dt.int32)
            s0_t = consts.tile([P, in_k], mybir.dt.float32)
            s2_t = consts.tile([P, mid_k], mybir.dt.float32)
            s4_t = consts.tile([P, out_k], mybir.dt.float32)
            bias_t = consts.tile([P, out_k], mybir.dt.float32)
            neg_half = consts.tile([P, 1], mybir.dt.float32)
            for t, d in (
                (mask_t, mask_d),
                (s0_t, s0_d),
                (s2_t, s2_d),
                (s4_t, s4_d),
                (bias_t, bias_d),
            ):
                nc.sync.dma_start(t[:], d[:])
            nc.vector.memset(neg_half[:], -0.5)

            def unpack_wT(bp_d, m, k_blocks):
                """Unpack 128 rows (block m) of a packed sign matrix and
                return its transposed [P, k_blocks, P] weight tile."""
                kb = k_blocks * P // 8  # bytes per row
                byt = wpipe.tile([P, kb], mybir.dt.int32, tag="bytes")
                nc.sync.dma_start(byt[:], bp_d[m * P : (m + 1) * P, :])
                w_nat = wpipe.tile([P, k_blocks * P], mybir.dt.float16, tag="wnat")
                for c0 in range(0, k_blocks * P, uch):
                    nb = uch // 8
                    b0 = c0 // 8
                    masked = wpipe.tile([P, uch], mybir.dt.int32, tag="masked")
                    in0 = byt[:, b0 : b0 + nb][:, :, None].broadcast_to([P, nb, 8])
                    in1 = mask_t[:][:, None, :].broadcast_to([P, nb, 8])
                    nc.vector.tensor_tensor(
                        masked[:].rearrange("p (b j) -> p b j", j=8),
                        in0,
                        in1,
                        mybir.AluOpType.bitwise_and,
                    )
                    nc.scalar.activation(
                        w_nat[:, c0 : c0 + uch],
                        masked[:],
                        Act.Sign,
                        bias=neg_half[:, 0:1],
                    )
                wT = wpipe.tile([P, k_blocks, P], mybir.dt.float16, tag="wT")
                nc.sync.dma_start_transpose(wT[:], w_nat[:])
                return wT

            # Warm the PE HAM clock gate with cheap junk matmuls while the
            # input pipeline fills, so the real stream starts at 2.4 GHz.
            junk = mask_t[:].bitcast(mybir.dt.float16)  # [P, 16] arbitrary bits
            warm_ps = psum.tile([P, 16], mybir.dt.float32, tag="warm")
            for _ in range(500):
                nc.tensor.matmul(warm_ps[:16, :], junk, junk, start=True, stop=True)

            # First weight block's unpack goes ahead of the x pipeline: it
            # only needs DVE/ACT, which are idle while DMA transposes x.
            pending_wT = unpack_wT(bp1_d, 0, in_k)

            # x.T in two batch halves: xH[h][p, k, r] = x[h*b/2 + r, 128k + p].
            # Whole-half transposes read DRAM contiguously; s0 scaling runs on
            # DVE (per-partition scalar) to keep ACT free for weight unpack.
            half = b // 2
            xH = []
            for h in range(2):
                xh = big.tile([P, in_k, half], mybir.dt.float16, tag=f"xT{h}")
                nc.sync.dma_start_transpose(xh[:], x_d[h * half : (h + 1) * half, :])
                for k in range(in_k):
                    nc.vector.tensor_scalar(
                        xh[:, k, :], xh[:, k, :], s0_t[:, k : k + 1], None,
                        mybir.AluOpType.mult,
                    )
                xH.append(xh)

            hT = big.tile([P, mid_k, b], mybir.dt.float16)

            # Unified block loop: GEMM1 (32 mid blocks) then GEMM2 (32 out
            # blocks), with the next block's weight unpack emitted before this
            # block's matmuls so ACT/DVE/DMA stay ahead of the PE.
            n_blocks = mid_k + out_k
            yt = None
            for j in range(n_blocks):
                wT = pending_wT
                if j + 1 < n_blocks:
                    if j + 1 < mid_k:
                        pending_wT = unpack_wT(bp1_d, j + 1, in_k)
                    else:
                        pending_wT = unpack_wT(bp3_d, j + 1 - mid_k, mid_k)
                if j < mid_k:  # GEMM1 block
                    m = j
                    for c in range(nbc):
                        ps = psum.tile([P, fd], mybir.dt.float32, tag="ps")
                        for k in range(in_k):
                            nc.tensor.matmul(
                                ps[:],
                                wT[:, k, :],
                                xH[c][:, k, :],
                                start=(k == 0),
                                stop=(k == in_k - 1),
                            )
                        nc.scalar.activation(
                            hT[:, m, c * fd : (c + 1) * fd],
                            ps[:],
                            Act.Copy,
                            scale=s2_t[:, m : m + 1],
                        )
                else:  # GEMM2 block
                    o = j - mid_k
                    yt = wpipe.tile([P, b], mybir.dt.float16, tag="yt")
                    for c in range(nbc):
                        ps = psum.tile([P, fd], mybir.dt.float32, tag="ps")
                        for k in range(mid_k):
                            nc.tensor.matmul(
                                ps[:],
                                wT[:, k, :],
                                hT[:, k, c * fd : (c + 1) * fd],
                                start=(k == 0),
                                stop=(k == mid_k - 1),
                            )
                        nc.scalar.activation(
                            yt[:, c * fd : (c + 1) * fd],
                            ps[:],
                            Act.Identity,
                            bias=bias_t[:, o : o + 1],
                            scale=s4_t[:, o : o + 1],
                        )
                    nc.sync.dma_start(yT_d[o * P : (o + 1) * P, :], yt[:])

    nc.compile()
    return nc


def make_in_maps(x, scaling0, bp1, scaling2, bp3, scaling4, bias, n_cores=N_CORES):
    b_full, in_ = x.shape
    mid = scaling2.shape[0]
    out = scaling4.shape[0]
    b = b_full // n_cores

    mask = (1 << (7 - np.arange(8, dtype=np.int32)))[None, :].repeat(P, 0)
    mask = np.ascontiguousarray(mask.astype(np.int32))

    def pcol(v):
        return np.ascontiguousarray(v.astype(np.float32).reshape(-1, P).T)

    shared = {
        "bp1": np.ascontiguousarray(bp1.reshape(mid, in_ // 8)),
        "bp3": np.ascontiguousarray(bp3.reshape(out, mid // 8)),
        "mask": mask,
        "s0": pcol(scaling0),
        "s2": pcol(scaling2),
        "s4": pcol(scaling4),
        "bias": pcol(bias),
    }
    return [
        {"x": np.ascontiguousarray(x[c * b : (c + 1) * b]), **shared}
        for c in range(n_cores)
    ]


_PROGRAM_CACHE = {}


def run(x, scaling0, bp1, scaling2, bp3, scaling4, bias, **spmd_kwargs):
    """Compile (cached) + run on 8 cores; returns (y, BassKernelResults)."""
    if "nc" not in _PROGRAM_CACHE:
        _PROGRAM_CACHE["nc"] = build_program()
    nc = _PROGRAM_CACHE["nc"]
    in_maps = make_in_maps(x, scaling0, bp1, scaling2, bp3, scaling4, bias)
    res = run_bass_kernel_spmd(nc, in_maps, core_ids=list(range(N_CORES)), **spmd_kwargs)
    b = x.shape[0] // N_CORES
    y = np.empty((x.shape[0], scaling4.shape[0]), dtype=np.float16)
    for c in range(N_CORES):
        y[c * b : (c + 1) * b] = res.results[c]["yT"].T
    return y, res


def kernel(x, scaling0, bp1, scaling2, bp3, scaling4, bias):
    y, _ = run(x, scaling0, bp1, scaling2, bp3, scaling4, bias)
    return y


# revision 8
# speedup vs baseline: 1.1223x; 1.1049x over previous
"""Trainium2 Bass kernel for DBFLinear:
    y = ((x * s0) @ unpack(bp1).T * s2) @ unpack(bp3).T * s4 + bias

Strategy: data-parallel over batch across 8 cores (weights replicated, no
collectives). Per core: unpack the bit-packed +/-1 weights on device
(DVE bitwise_and + ACT Sign), transpose weight blocks with the DMA xbar,
run both GEMMs weight-stationary (fp16, fp32 PSUM accumulation). scaling0
is folded into the unpacked W1 (+/-s0 is exact in fp16), scaling2 into the
h eviction, scaling4+bias into the y eviction — all per-partition ACT ops.
The device emits y.T per batch shard; the host transposes while unsharding.
"""

import sys

import numpy as np

sys.path.insert(0, "/opt/trn_rl_repo")

import concourse.bass as bass
import concourse.mybir as mybir
import concourse.tile as tile
from concourse import bacc
from concourse.bass_utils import run_bass_kernel_spmd

N_CORES = 8
B_FULL, IN, MID, OUT = 8192, 4096, 4096, 4096
P = 128
FD = 512  # matmul moving-operand free dim (1 PSUM bank of fp32)
QCH = 1024  # unpack quarter width (weight elements per DVE/ACT op)
N_WARM = 500  # HAM warm-up matmuls


def build_program(b=B_FULL // N_CORES, in_=IN, mid=MID, out=OUT):
    """Build the per-core Bass program. Returns the Bass object."""
    in_k, mid_k, out_k = in_ // P, mid // P, out // P
    nbc = 2  # batch processed as two halves
    fd = b // nbc
    assert fd <= FD, (b, fd)
    uch = min(QCH, in_, mid)

    nc = bacc.Bacc(num_devices=N_CORES)
    x_d = nc.dram_tensor("x", [b, in_], mybir.dt.float16, kind="ExternalInput")
    bp1_d = nc.dram_tensor("bp1", [mid, in_ // 8], mybir.dt.int32, kind="ExternalInput")
    bp3_d = nc.dram_tensor("bp3", [out, mid // 8], mybir.dt.int32, kind="ExternalInput")
    mask_d = nc.dram_tensor("mask", [P, 8], mybir.dt.int32, kind="ExternalInput")
    s0r_d = nc.dram_tensor("s0rep", [P, in_], mybir.dt.float16, kind="ExternalInput")
    s2_d = nc.dram_tensor("s2", [P, mid_k], mybir.dt.float32, kind="ExternalInput")
    s4_d = nc.dram_tensor("s4", [P, out_k], mybir.dt.float32, kind="ExternalInput")
    bias_d = nc.dram_tensor("bias", [P, out_k], mybir.dt.float32, kind="ExternalInput")
    yT_d = nc.dram_tensor("yT", [out, b], mybir.dt.float16, kind="ExternalOutput")

    Act = mybir.ActivationFunctionType

    with tile.TileContext(nc) as tc:
        with (
            tc.tile_pool(name="big", bufs=1) as big,
            tc.tile_pool(name="consts", bufs=1) as consts,
            tc.tile_pool(name="wpipe", bufs=2) as wpipe,
            tc.tile_pool(name="psum", bufs=4, space="PSUM") as psum,
        ):
            mask_t = consts.tile([P, 8], mybir.dt.int32)
            s0r_t = consts.tile([P, in_], mybir.dt.float16)
            s2_t = consts.tile([P, mid_k], mybir.dt.float32)
            s4_t = consts.tile([P, out_k], mybir.dt.float32)
            bias_t = consts.tile([P, out_k], mybir.dt.float32)
            neg_half = consts.tile([P, 1], mybir.dt.float32)
            for t, d in (
                (mask_t, mask_d),
                (s0r_t, s0r_d),
                (s2_t, s2_d),
                (s4_t, s4_d),
                (bias_t, bias_d),
            ):
                nc.gpsimd.dma_start(t[:], d[:])
            nc.vector.memset(neg_half[:], -0.5)

            # Warm the PE HAM clock gate with cheap junk matmuls while the
            # input pipeline fills, so the real stream starts at 2.4 GHz.
            junk = mask_t[:].bitcast(mybir.dt.float16)  # [P, 16] arbitrary bits
            warm_ps = psum.tile([P, 16], mybir.dt.float32, tag="warm")
            for _ in range(N_WARM):
                nc.tensor.matmul(warm_ps[:16, :], junk, junk, start=True, stop=True)

            def unpack_wT(bp_d, m, k_blocks, scale_s0):
                """Unpack 128 rows (block m) of a packed sign matrix into its
                transposed [P, k_blocks, P] weight tile, quarter by quarter.
                scale_s0: also multiply by the replicated scaling0 row."""
                kb = k_blocks * P // 8  # bytes per row
                byt = wpipe.tile([P, kb], mybir.dt.int32, tag="bytes", bufs=3)
                nc.gpsimd.dma_start(byt[:], bp_d[m * P : (m + 1) * P, :])
                wT = wpipe.tile([P, k_blocks, P], mybir.dt.float16, tag="wT", bufs=3)
                for c0 in range(0, k_blocks * P, uch):
                    nb = uch // 8
                    b0 = c0 // 8
                    masked = wpipe.tile([P, uch], mybir.dt.int32, tag="masked", bufs=3)
                    in0 = byt[:, b0 : b0 + nb][:, :, None].broadcast_to([P, nb, 8])
                    in1 = mask_t[:][:, None, :].broadcast_to([P, nb, 8])
                    nc.vector.tensor_tensor(
                        masked[:].rearrange("p (b j) -> p b j", j=8),
                        in0,
                        in1,
                        mybir.AluOpType.bitwise_and,
                    )
                    wq = wpipe.tile([P, uch], mybir.dt.float16, tag="wnat", bufs=4)
                    nc.scalar.activation(
                        wq[:], masked[:], Act.Sign, bias=neg_half[:, 0:1]
                    )
                    if scale_s0:
                        nc.vector.tensor_tensor(
                            wq[:], wq[:], s0r_t[:, c0 : c0 + uch],
                            mybir.AluOpType.mult,
                        )
                    nc.sync.dma_start_transpose(
                        wT[:, c0 // P : (c0 + uch) // P, :], wq[:]
                    )
                return wT

            # x.T in two batch halves: xH[h][p, k, r] = x[h*b/2 + r, 128k + p].
            # Whole-half transposes read DRAM contiguously; no scaling needed
            # (scaling0 lives in W1).
            half = b // 2
            xH = []
            for h in range(2):
                xh = big.tile([P, in_k, half], mybir.dt.float16, tag=f"xT{h}")
                xH.append(xh)

            nc.sync.dma_start_transpose(xH[0][:], x_d[0:half, :])
            pend = [unpack_wT(bp1_d, 0, in_k, True)]
            nc.sync.dma_start_transpose(xH[1][:], x_d[half : 2 * half, :])
            pend.append(unpack_wT(bp1_d, 1, in_k, True))

            hT = big.tile([P, mid_k, b], mybir.dt.float16)

            # Unified block loop: GEMM1 (mid blocks) then GEMM2 (out blocks),
            # with weight unpack prefetched two blocks ahead so DVE/ACT/xbar
            # stay ahead of the PE.
            n_blocks = mid_k + out_k
            for j in range(n_blocks):
                wT = pend.pop(0)
                if j + 2 < n_blocks:
                    if j + 2 < mid_k:
                        pend.append(unpack_wT(bp1_d, j + 2, in_k, True))
                    else:
                        pend.append(unpack_wT(bp3_d, j + 2 - mid_k, mid_k, False))
                if j < mid_k:  # GEMM1 block
                    m = j
                    for c in range(nbc):
                        ps = psum.tile([P, fd], mybir.dt.float32, tag="ps")
                        for k in range(in_k):
                            nc.tensor.matmul(
                                ps[:],
                                wT[:, k, :],
                                xH[c][:, k, :],
                                start=(k == 0),
                                stop=(k == in_k - 1),
                            )
                        nc.scalar.activation(
                            hT[:, m, c * fd : (c + 1) * fd],
                            ps[:],
                            Act.Copy,
                            scale=s2_t[:, m : m + 1],
                        )
                else:  # GEMM2 block
                    o = j - mid_k
                    yt = wpipe.tile([P, b], mybir.dt.float16, tag="yt")
                    for c in range(nbc):
                        ps = psum.tile([P, fd], mybir.dt.float32, tag="ps")
                        for k in range(mid_k):
                            nc.tensor.matmul(
                                ps[:],
                                wT[:, k, :],
                                hT[:, k, c * fd : (c + 1) * fd],
                                start=(k == 0),
                                stop=(k == mid_k - 1),
                            )
                        nc.scalar.activation(
                            yt[:, c * fd : (c + 1) * fd],
                            ps[:],
                            Act.Identity,
                            bias=bias_t[:, o : o + 1],
                            scale=s4_t[:, o : o + 1],
                        )
                    nc.gpsimd.dma_start(yT_d[o * P : (o + 1) * P, :], yt[:])

    nc.compile()
    return nc


def make_in_maps(x, scaling0, bp1, scaling2, bp3, scaling4, bias, n_cores=N_CORES):
    b_full, in_ = x.shape
    mid = scaling2.shape[0]
    out = scaling4.shape[0]
    b = b_full // n_cores

    mask = (1 << (7 - np.arange(8, dtype=np.int32)))[None, :].repeat(P, 0)
    mask = np.ascontiguousarray(mask.astype(np.int32))

    def pcol(v):
        return np.ascontiguousarray(v.astype(np.float32).reshape(-1, P).T)

    shared = {
        "bp1": np.ascontiguousarray(bp1.reshape(mid, in_ // 8)),
        "bp3": np.ascontiguousarray(bp3.reshape(out, mid // 8)),
        "mask": mask,
        "s0rep": np.ascontiguousarray(
            np.broadcast_to(scaling0.astype(np.float16)[None, :], (P, in_))
        ),
        "s2": pcol(scaling2),
        "s4": pcol(scaling4),
        "bias": pcol(bias),
    }
    return [
        {"x": np.ascontiguousarray(x[c * b : (c + 1) * b]), **shared}
        for c in range(n_cores)
    ]


_PROGRAM_CACHE = {}


def run(x, scaling0, bp1, scaling2, bp3, scaling4, bias, **spmd_kwargs):
    """Compile (cached) + run on 8 cores; returns (y, BassKernelResults)."""
    if "nc" not in _PROGRAM_CACHE:
        _PROGRAM_CACHE["nc"] = build_program()
    nc = _PROGRAM_CACHE["nc"]
    in_maps = make_in_maps(x, scaling0, bp1, scaling2, bp3, scaling4, bias)
    res = run_bass_kernel_spmd(nc, in_maps, core_ids=list(range(N_CORES)), **spmd_kwargs)
    b = x.shape[0] // N_CORES
    y = np.empty((x.shape[0], scaling4.shape[0]), dtype=np.float16)
    for c in range(N_CORES):
        y[c * b : (c + 1) * b] = res.results[c]["yT"].T
    return y, res


def kernel(x, scaling0, bp1, scaling2, bp3, scaling4, bias):
    y, _ = run(x, scaling0, bp1, scaling2, bp3, scaling4, bias)
    return y


# revision 11
# speedup vs baseline: 1.1512x; 1.0258x over previous
"""Trainium2 Bass kernel for DBFLinear:
    y = ((x * s0) @ unpack(bp1).T * s2) @ unpack(bp3).T * s4 + bias

Strategy: data-parallel over batch across 8 cores (weights replicated, no
collectives). Per core: unpack the bit-packed +/-1 weights on device
(DVE bitwise_and + ACT Sign), transpose weight blocks with the DMA xbar,
run both GEMMs weight-stationary (fp16, fp32 PSUM accumulation). scaling0
is folded into the unpacked W1 (+/-s0 is exact in fp16), scaling2 into the
h eviction, scaling4+bias into the y eviction — all per-partition ACT ops.
The device emits y.T per batch shard; the host transposes while unsharding.
"""

import sys

import numpy as np

sys.path.insert(0, "/opt/trn_rl_repo")

import concourse.bass as bass
import concourse.mybir as mybir
import concourse.tile as tile
from concourse import bacc
from concourse.bass_utils import run_bass_kernel_spmd

N_CORES = 8
B_FULL, IN, MID, OUT = 8192, 4096, 4096, 4096
P = 128
FD = 512  # matmul moving-operand free dim (1 PSUM bank of fp32)
QCH = 1024  # unpack quarter width (weight elements per DVE/ACT op)
N_WARM = 500  # HAM warm-up matmuls


def build_program(b=B_FULL // N_CORES, in_=IN, mid=MID, out=OUT):
    """Build the per-core Bass program. Returns the Bass object."""
    in_k, mid_k, out_k = in_ // P, mid // P, out // P
    nbc = 2  # batch processed as two halves
    fd = b // nbc
    assert fd <= FD, (b, fd)
    uch = min(QCH, in_, mid)

    nc = bacc.Bacc(num_devices=N_CORES)
    x_d = nc.dram_tensor("x", [b, in_], mybir.dt.float16, kind="ExternalInput")
    bp1_d = nc.dram_tensor("bp1", [mid, in_ // 8], mybir.dt.int32, kind="ExternalInput")
    bp3_d = nc.dram_tensor("bp3", [out, mid // 8], mybir.dt.int32, kind="ExternalInput")
    mask_d = nc.dram_tensor("mask", [P, 8], mybir.dt.int32, kind="ExternalInput")
    s0r_d = nc.dram_tensor("s0rep", [P, in_], mybir.dt.float16, kind="ExternalInput")
    s2_d = nc.dram_tensor("s2", [P, mid_k], mybir.dt.float32, kind="ExternalInput")
    s4_d = nc.dram_tensor("s4", [P, out_k], mybir.dt.float32, kind="ExternalInput")
    bias_d = nc.dram_tensor("bias", [P, out_k], mybir.dt.float32, kind="ExternalInput")
    yT_d = nc.dram_tensor("yT", [out, b], mybir.dt.float16, kind="ExternalOutput")

    Act = mybir.ActivationFunctionType

    with tile.TileContext(nc) as tc:
        with (
            tc.tile_pool(name="big", bufs=1) as big,
            tc.tile_pool(name="consts", bufs=1) as consts,
            tc.tile_pool(name="wpipe", bufs=2) as wpipe,
            tc.tile_pool(name="psum", bufs=4, space="PSUM") as psum,
        ):
            mask_t = consts.tile([P, 8], mybir.dt.int32)
            s0r_t = consts.tile([P, in_], mybir.dt.float16)
            s2_t = consts.tile([P, mid_k], mybir.dt.float32)
            s4_t = consts.tile([P, out_k], mybir.dt.float32)
            bias_t = consts.tile([P, out_k], mybir.dt.float32)
            neg_half = consts.tile([P, 1], mybir.dt.float32)
            for t, d in (
                (mask_t, mask_d),
                (s0r_t, s0r_d),
                (s2_t, s2_d),
                (s4_t, s4_d),
                (bias_t, bias_d),
            ):
                nc.gpsimd.dma_start(t[:], d[:])
            nc.vector.memset(neg_half[:], -0.5)

            # Warm the PE HAM clock gate with cheap junk matmuls while the
            # input pipeline fills, so the real stream starts at 2.4 GHz.
            junk = mask_t[:].bitcast(mybir.dt.float16)  # [P, 16] arbitrary bits
            warm_ps = psum.tile([P, 16], mybir.dt.float32, tag="warm")
            for _ in range(N_WARM):
                nc.tensor.matmul(warm_ps[:16, :], junk, junk, start=True, stop=True)

            def load_bytes(bp_d, m, k_blocks):
                kb = k_blocks * P // 8  # bytes per row
                byt = wpipe.tile([P, kb], mybir.dt.int32, tag="bytes", bufs=2)
                nc.sync.dma_start(byt[:], bp_d[m * P : (m + 1) * P, :])
                return byt

            def unpack_quarters(byt, k_blocks, scale_s0):
                """Unpack a loaded 128-row byte block into its transposed
                [P, k_blocks, P] weight tile, quarter by quarter.
                scale_s0: also multiply by the replicated scaling0 row."""
                wT = wpipe.tile([P, k_blocks, P], mybir.dt.float16, tag="wT", bufs=4)
                for c0 in range(0, k_blocks * P, uch):
                    nb = uch // 8
                    b0 = c0 // 8
                    masked = wpipe.tile([P, uch], mybir.dt.int32, tag="masked", bufs=2)
                    in0 = byt[:, b0 : b0 + nb][:, :, None].broadcast_to([P, nb, 8])
                    in1 = mask_t[:][:, None, :].broadcast_to([P, nb, 8])
                    nc.vector.tensor_tensor(
                        masked[:].rearrange("p (b j) -> p b j", j=8),
                        in0,
                        in1,
                        mybir.AluOpType.bitwise_and,
                    )
                    wq = wpipe.tile([P, uch], mybir.dt.float16, tag="wnat", bufs=4)
                    nc.scalar.activation(
                        wq[:], masked[:], Act.Sign, bias=neg_half[:, 0:1]
                    )
                    if scale_s0:
                        nc.vector.tensor_tensor(
                            wq[:], wq[:], s0r_t[:, c0 : c0 + uch],
                            mybir.AluOpType.mult,
                        )
                    nc.sync.dma_start_transpose(
                        wT[:, c0 // P : (c0 + uch) // P, :], wq[:]
                    )
                return wT

            def unpack_wT(bp_d, m, k_blocks, scale_s0):
                return unpack_quarters(load_bytes(bp_d, m, k_blocks), k_blocks, scale_s0)

            # x.T in two batch halves: xH[h][p, k, r] = x[h*b/2 + r, 128k + p].
            # Band-split whole-half transposes read DRAM contiguously; no
            # scaling needed (scaling0 lives in W1).
            half = b // 2
            xH = [
                big.tile([P, in_k, half], mybir.dt.float16, tag=f"xT{h}", name=f"xh{h}")
                for h in range(2)
            ]

            def x_bands(h):
                kb2 = in_k // 2
                for band in range(2):
                    nc.sync.dma_start_transpose(
                        xH[h][:, band * kb2 : (band + 1) * kb2, :],
                        x_d[h * half : (h + 1) * half, band * kb2 * P : (band + 1) * kb2 * P],
                    )

            # Startup: prefetch byte blocks, transpose the first x half, then
            # unpack the first START_BLOCKS weight blocks, then the second x
            # half. The PE runs c0 passes of blocks 0..3 against the first x
            # half while the second is still transposing.
            SB = min(4, mid_k)
            byts = [load_bytes(bp1_d, m, in_k) for m in range(min(2, SB))]
            x_bands(0)
            byts += [load_bytes(bp1_d, m, in_k) for m in range(2, SB)]
            wTs = [unpack_quarters(byts[m], in_k, True) for m in range(SB)]
            x_bands(1)

            hT = big.tile([P, mid_k, b], mybir.dt.float16)

            def g1_pass(m, wT, c):
                ps = psum.tile([P, fd], mybir.dt.float32, tag="ps")
                for k in range(in_k):
                    nc.tensor.matmul(
                        ps[:],
                        wT[:, k, :],
                        xH[c][:, k, :],
                        start=(k == 0),
                        stop=(k == in_k - 1),
                    )
                nc.scalar.activation(
                    hT[:, m, c * fd : (c + 1) * fd],
                    ps[:],
                    Act.Copy,
                    scale=s2_t[:, m : m + 1],
                )

            # c-major startup over the first SB blocks
            for c in range(nbc):
                for m in range(SB):
                    g1_pass(m, wTs[m], c)

            # Unified steady loop: GEMM1 blocks SB.., then GEMM2 blocks, with
            # weight unpack prefetched two blocks ahead.
            n_blocks = mid_k + out_k

            def mk(jj):
                if jj >= n_blocks:
                    return None
                if jj < mid_k:
                    return unpack_wT(bp1_d, jj, in_k, True)
                return unpack_wT(bp3_d, jj - mid_k, mid_k, False)

            pend = [mk(SB), mk(SB + 1)]
            for j in range(SB, n_blocks):
                wT = pend.pop(0)
                pend.append(mk(j + 2))
                if j < mid_k:  # GEMM1 block
                    for c in range(nbc):
                        g1_pass(j, wT, c)
                else:  # GEMM2 block
                    o = j - mid_k
                    yt = wpipe.tile([P, b], mybir.dt.float16, tag="yt", bufs=1)
                    for c in range(nbc):
                        ps = psum.tile([P, fd], mybir.dt.float32, tag="ps")
                        for k in range(mid_k):
                            nc.tensor.matmul(
                                ps[:],
                                wT[:, k, :],
                                hT[:, k, c * fd : (c + 1) * fd],
                                start=(k == 0),
                                stop=(k == mid_k - 1),
                            )
                        nc.scalar.activation(
                            yt[:, c * fd : (c + 1) * fd],
                            ps[:],
                            Act.Identity,
                            bias=bias_t[:, o : o + 1],
                            scale=s4_t[:, o : o + 1],
                        )
                    nc.sync.dma_start(yT_d[o * P : (o + 1) * P, :], yt[:])

    nc.compile()
    return nc


def make_in_maps(x, scaling0, bp1, scaling2, bp3, scaling4, bias, n_cores=N_CORES):
    b_full, in_ = x.shape
    mid = scaling2.shape[0]
    out = scaling4.shape[0]
    b = b_full // n_cores

    mask = (1 << (7 - np.arange(8, dtype=np.int32)))[None, :].repeat(P, 0)
    mask = np.ascontiguousarray(mask.astype(np.int32))

    def pcol(v):
        return np.ascontiguousarray(v.astype(np.float32).reshape(-1, P).T)

    shared = {
        "bp1": np.ascontiguousarray(bp1.reshape(mid, in_ // 8)),
        "bp3": np.ascontiguousarray(bp3.reshape(out, mid // 8)),
        "mask": mask,
        "s0rep": np.ascontiguousarray(
            np.broadcast_to(scaling0.astype(np.float16)[None, :], (P, in_))
        ),
        "s2": pcol(scaling2),
        "s4": pcol(scaling4),
        "bias": pcol(bias),
    }
    return [
        {"x": np.ascontiguousarray(x[c * b : (c + 1) * b]), **shared}
        for c in range(n_cores)
    ]


_PROGRAM_CACHE = {}


def run(x, scaling0, bp1, scaling2, bp3, scaling4, bias, **spmd_kwargs):
    """Compile (cached) + run on 8 cores; returns (y, BassKernelResults)."""
    if "nc" not in _PROGRAM_CACHE:
        _PROGRAM_CACHE["nc"] = build_program()
    nc = _PROGRAM_CACHE["nc"]
    in_maps = make_in_maps(x, scaling0, bp1, scaling2, bp3, scaling4, bias)
    res = run_bass_kernel_spmd(nc, in_maps, core_ids=list(range(N_CORES)), **spmd_kwargs)
    b = x.shape[0] // N_CORES
    y = np.empty((x.shape[0], scaling4.shape[0]), dtype=np.float16)
    for c in range(N_CORES):
        y[c * b : (c + 1) * b] = res.results[c]["yT"].T
    return y, res


def kernel(x, scaling0, bp1, scaling2, bp3, scaling4, bias):
    y, _ = run(x, scaling0, bp1, scaling2, bp3, scaling4, bias)
    return y
